# revision 16
# baseline (speedup 1.0000x reference)
"""Trainium2 Bass kernel for nn_CMFA (dense_transformer, seq_len=1 cross-attention).

Math notes (exact simplifications vs the reference):
  - softmax over a single key is exactly 1.0, so mha(q,k,v) reduces to the
    v-projection chain: (v @ Wv.T + bv) @ Wo.T + bo. q/k projections are dead.
  - Wv -> Wo -> fi2 is a linear chain, folded on the host:
      V = v1 @ Mv.T + i_ @ fi2.T + bV   with Mv = fi2 @ (Wo @ Wv)
      T = v2 @ Mt.T + t_ @ ft2.T + bT   with Mt = ft2 @ (Wo @ Wv)

Precision strategy (measured: bf16 matmul ~216ns per 128K x 512N block; fp8e4
DoubleRow contracts 256 rows per pass at the same per-column rate = 2x FLOPs):
  - bf16 (error-critical): fi1 (error amplified through the residual), ft1,
    and the V residual half (i_ @ fi2 dominates output magnitude).
  - fp8e4 + DoubleRow: ci1, ct1, the v1/v2 halves, and the T residual half.
    Simulated + measured end-to-end rel err ~1.1e-2 vs the 2e-2 gate.
  - fp8 frames are chosen so every fp8 activation is exactly
    e4m3(max(psum + bias, 0)) -- a single vector-engine tensor_scalar op
    (e4m3 relative precision is scale-free; frames only bound the range):
      i_, t_ in frame 1.0; v1 in frame sWc1 (= ci1 weight scale); v2 in sWc2.
    Dequant happens in the next layer's weight scale / output activation.
  - Engine split: PE matmuls; VECTOR makes all fp8 activations; SCALAR makes
    the bf16 i_ copy and the final V/T psum->f32 outputs (scale+bias native).

Device layout: activations feature-major ([feat, batch]); every matmul
contracts over the partition dim.  Pure data parallel across 8 cores.
"""

import numpy as np
import ml_dtypes

B, IMG, TAB, HID = 32768, 2048, 128, 512
NCORES = 8
BS = B // NCORES  # rows per core
NT = 512          # batch-tile (matmul moving/free dim)

# fp8 frames for device-produced activations (range-driven; e4m3 max is 240
# on TRN).  True maxima: i_ 5.65, t_ 1.66, v1 1.66, v2 0.46.
S_WC1 = 96.0      # ci1 weight scale == v1 frame (v1 max * 96 = 160)
S_WC2 = 64.0      # ct1 weight scale == v2 frame (v2 max * 64 = 30)

_CACHE = {}

E4 = ml_dtypes.float8_e4m3   # TRN-compatible e4m3: max normal 240
BF = ml_dtypes.bfloat16


def _pack_bf16(WT: np.ndarray, K: int, M: int) -> np.ndarray:
    """[K*128, M*128] -> [128, K*M*128] bf16, col ((k*M+m)*128+j) = WT[k*128+p, m*128+j]."""
    out = WT.reshape(K, 128, M, 128).transpose(1, 0, 2, 3).reshape(128, K * M * 128)
    return np.ascontiguousarray(out.astype(np.float32)).astype(BF)


def _pack_fp8(WTs: np.ndarray, kd: int) -> np.ndarray:
    """Scaled [kd*256, 512] -> [128, kd*1024] e4m3 DoubleRow layout:
    [p, k, i, m*128+j] = WTs[(2k+i)*128+p, m*128+j]."""
    q = WTs.astype(E4)
    assert np.isfinite(q.astype(np.float32)).all(), "fp8 overflow in weight pack"
    out = q.reshape(kd, 2, 128, 4, 128).transpose(2, 0, 1, 3, 4).reshape(128, kd * 1024)
    return np.ascontiguousarray(out)


def _host_pack(inp: dict):
    f8d = np.float64
    def g(n):
        return np.asarray(inp[n], dtype=np.float32)

    fi1_w, fi1_b = g("fi1_w"), g("fi1_b")
    ft1_w, ft1_b = g("ft1_w"), g("ft1_b")
    ci1_w, ci1_b = g("ci1_w"), g("ci1_b")
    ct1_w, ct1_b = g("ct1_w"), g("ct1_b")

    def fold(wv, bv, wo, bo, f_w, f_b):
        Wvo = wo.astype(f8d) @ wv.astype(f8d)
        bvo = wo.astype(f8d) @ bv.astype(f8d) + bo.astype(f8d)
        M = (f_w.astype(f8d) @ Wvo).astype(np.float32)
        bias = (f_w.astype(f8d) @ bvo + f_b.astype(f8d)).astype(np.float32)
        return M, f_w, bias

    Mv, fi2, bV = fold(g("aV_wv"), g("aV_bv"), g("aV_wo"), g("aV_bo"),
                       g("fi2_w"), g("fi2_b"))
    Mt, ft2, bT = fold(g("aT_wv"), g("aT_bv"), g("aT_wo"), g("aT_bo"),
                       g("ft2_w"), g("ft2_b"))

    sMv = 200.0 / np.abs(Mv).max()
    sF2 = 200.0 / np.abs(ft2).max()     # t_ frame is 1.0 -> T psum frame = sF2
    sMt = sF2 / S_WC2                   # v2 frame sWc2: match T psum frame
    assert np.abs(Mt).max() * sMt < 220.0

    weights = {
        "w_fi1": _pack_bf16(np.ascontiguousarray(fi1_w.T), 16, 4),
        "w_ft1": _pack_bf16(np.ascontiguousarray(ft1_w.T), 1, 4),
        "w_fi2s": _pack_bf16(np.ascontiguousarray(fi2.T) * (S_WC1 * sMv), 4, 4),
        "w_ci1": _pack_fp8(np.ascontiguousarray(ci1_w.T) * S_WC1, 2),
        "w_ct1": _pack_fp8(np.ascontiguousarray(ct1_w.T) * S_WC2, 2),
        "w_Vv1": _pack_fp8(np.ascontiguousarray(Mv.T) * sMv, 2),
        "w_Tv2": _pack_fp8(np.ascontiguousarray(Mt.T) * sMt, 2),
        "w_Tt": _pack_fp8(np.ascontiguousarray(ft2.T) * sF2, 2),
    }
    cols = []
    for b in (fi1_b, ft1_b, ci1_b * S_WC1, ct1_b * S_WC2, bV, bT):
        for m in range(4):
            cols.append(b[128 * m:128 * (m + 1)])
    weights["bias"] = np.ascontiguousarray(np.stack(cols, axis=1), dtype=np.float32)

    scales = {
        "V": 1.0 / (S_WC1 * sMv),       # psum(V) -> true scale
        "T": 1.0 / sF2,                 # psum(T) -> true scale
    }
    return weights, scales


def _build_nc(bs: int, scales: dict):
    import concourse.tile as tile
    from concourse import bacc, mybir

    f32 = mybir.dt.float32
    bf16 = mybir.dt.bfloat16
    f8 = mybir.dt.float8e4
    DR = mybir.MatmulPerfMode.DoubleRow
    Relu = mybir.ActivationFunctionType.Relu
    Ident = mybir.ActivationFunctionType.Identity
    ADD = mybir.AluOpType.add
    MAX = mybir.AluOpType.max
    ntiles = bs // NT

    nc = bacc.Bacc("TRN2", target_bir_lowering=False, debug=False)

    iT_d = nc.dram_tensor("iT", [IMG, bs], bf16, kind="ExternalInput").ap()
    tT_d = nc.dram_tensor("tT", [TAB, bs], bf16, kind="ExternalInput").ap()
    w_fi1_d = nc.dram_tensor("w_fi1", [128, 16 * 512], bf16, kind="ExternalInput").ap()
    w_ft1_d = nc.dram_tensor("w_ft1", [128, 512], bf16, kind="ExternalInput").ap()
    w_fi2s_d = nc.dram_tensor("w_fi2s", [128, 4 * 512], bf16, kind="ExternalInput").ap()
    w_ci1_d = nc.dram_tensor("w_ci1", [128, 2048], f8, kind="ExternalInput").ap()
    w_ct1_d = nc.dram_tensor("w_ct1", [128, 2048], f8, kind="ExternalInput").ap()
    w_Vv1_d = nc.dram_tensor("w_Vv1", [128, 2048], f8, kind="ExternalInput").ap()
    w_Tv2_d = nc.dram_tensor("w_Tv2", [128, 2048], f8, kind="ExternalInput").ap()
    w_Tt_d = nc.dram_tensor("w_Tt", [128, 2048], f8, kind="ExternalInput").ap()
    bias_d = nc.dram_tensor("bias", [128, 24], f32, kind="ExternalInput").ap()
    out_d = nc.dram_tensor("outT", [2 * HID, bs], bf16, kind="ExternalOutput").ap()

    with tile.TileContext(nc) as tc:
        with (
            tc.tile_pool(name="w", bufs=1) as wpool,
            tc.tile_pool(name="x", bufs=2) as xpool,
            tc.tile_pool(name="h", bufs=2) as hpool,
            tc.tile_pool(name="o", bufs=8) as opool,
            tc.tile_pool(name="ps", bufs=8, space="PSUM") as pspool,
        ):
            wf1 = wpool.tile([128, 16, 512], bf16, name="wf1")
            wt1 = wpool.tile([128, 512], bf16, name="wt1")
            wf2 = wpool.tile([128, 4, 512], bf16, name="wf2")
            wc1 = wpool.tile([128, 2, 2, 512], f8, name="wc1")
            wc2 = wpool.tile([128, 2, 2, 512], f8, name="wc2")
            wV1 = wpool.tile([128, 2, 2, 512], f8, name="wV1")
            wT2 = wpool.tile([128, 2, 2, 512], f8, name="wT2")
            wTt = wpool.tile([128, 2, 2, 512], f8, name="wTt")
            bt = wpool.tile([128, 24], f32, name="bias_t")

            def bcol(s, m):
                c = 4 * s + m
                return bt[:, c:c + 1]

            def xload(n, nsplit=1):
                c0 = n * NT
                xt = xpool.tile([128, 16, NT], bf16, tag="x", name=f"x_{n}")
                kc = 16 // nsplit
                for s in range(nsplit):
                    nc.sync.dma_start(
                        xt[:, s * kc:(s + 1) * kc, :],
                        iT_d[s * kc * 128:(s + 1) * kc * 128, c0:c0 + NT].rearrange(
                            "(k p) n -> p k n", p=128))
                return xt

            # preamble: tile 0 finely split so fi1 can start on the first chunks
            nc.sync.dma_start(bt[:], bias_d[:])
            nc.sync.dma_start(wt1[:], w_ft1_d[:])
            nc.sync.dma_start(wf1[:, 0:4, :],
                              w_fi1_d[:, 0:4 * 512].rearrange("p (k n) -> p k n", k=4))
            x_cur = xload(0, nsplit=8)
            nc.sync.dma_start(wf1[:, 4:16, :],
                              w_fi1_d[:, 4 * 512:].rearrange("p (k n) -> p k n", k=12))
            xt_cur = xpool.tile([128, NT], bf16, tag="xt", bufs=2, name="xt_0")
            nc.sync.dma_start(xt_cur[:], tT_d[:, 0:NT])
            for wtile, dram in [(wc1, w_ci1_d), (wc2, w_ct1_d), (wV1, w_Vv1_d),
                                (wT2, w_Tv2_d), (wTt, w_Tt_d)]:
                nc.sync.dma_start(
                    wtile[:], dram[:].rearrange("p (a t n) -> p a t n", a=2, t=2))
            nc.sync.dma_start(wf2[:], w_fi2s_d[:].rearrange("p (a n) -> p a n", a=4))

            for n in range(ntiles):
                c0 = n * NT
                psA = [pspool.tile([128, NT], f32, tag="ps", name=f"psA_{n}_{m}")
                       for m in range(4)]
                psB = [pspool.tile([128, NT], f32, tag="ps", name=f"psB_{n}_{m}")
                       for m in range(4)]
                i_b = hpool.tile([128, 4, NT], bf16, tag="i_b", name=f"i_b_{n}")
                i_8 = hpool.tile([128, 2, 2, NT], f8, tag="i_8", name=f"i_8_{n}")
                t_8 = hpool.tile([128, 2, 2, NT], f8, tag="t_8", name=f"t_8_{n}")

                # ---- fi1 (bf16, psA); ft1 (psB) slotted mid-loop ----
                # tile 0 iterates k-outer so compute paces the initial x DMA;
                # steady-state tiles iterate m-outer so psA chunks retire
                # staggered for the downstream activations.
                def ft1_mms():
                    for mm in range(4):
                        nc.tensor.matmul(psB[mm][:], wt1[:, 128 * mm:128 * (mm + 1)],
                                         xt_cur[:], start=True, stop=True)
                    for mm in range(4):
                        nc.scalar.activation(t_8[:, mm // 2, mm % 2, :],
                                             psB[mm][:], Relu, bias=bcol(1, mm))
                if n == 0:
                    for k in range(16):
                        for m in range(4):
                            nc.tensor.matmul(psA[m][:],
                                             wf1[:, k, 128 * m:128 * (m + 1)],
                                             x_cur[:, k, :], start=k == 0, stop=k == 15)
                        if k == 7:
                            ft1_mms()
                    for m in range(4):
                        nc.scalar.activation(i_8[:, m // 2, m % 2, :], psA[m][:], Relu,
                                             bias=bcol(0, m))
                        nc.scalar.activation(i_b[:, m, :], psA[m][:], Relu,
                                             bias=bcol(0, m))
                else:
                    for m in range(4):
                        for k in range(16):
                            nc.tensor.matmul(psA[m][:],
                                             wf1[:, k, 128 * m:128 * (m + 1)],
                                             x_cur[:, k, :], start=k == 0, stop=k == 15)
                        nc.scalar.activation(i_8[:, m // 2, m % 2, :], psA[m][:], Relu,
                                             bias=bcol(0, m))
                        nc.scalar.activation(i_b[:, m, :], psA[m][:], Relu,
                                             bias=bcol(0, m))
                        if m == 1:
                            ft1_mms()
                # prefetch next tile's inputs (early in Sync program order)
                if n + 1 < ntiles:
                    x_nxt = xload(n + 1)
                    xt_nxt = xpool.tile([128, NT], bf16, tag="xt", bufs=2,
                                        name=f"xt_{n + 1}")
                    nc.sync.dma_start(xt_nxt[:], tT_d[:, c0 + NT:c0 + 2 * NT])

                # ---- ci1 (fp8 DR): v1 in frame S_WC1 ----
                psC = [pspool.tile([128, NT], f32, tag="ps", name=f"psC_{n}_{m}")
                       for m in range(4)]
                v1_8 = hpool.tile([128, 2, 2, NT], f8, tag="v1", name=f"v1_{n}")
                for k in range(2):
                    for m in range(4):
                        nc.tensor.matmul(psC[m][:], wc1[:, k, :, 128 * m:128 * (m + 1)],
                                         i_8[:, k, :, :], start=k == 0, stop=k == 1,
                                         perf_mode=DR)
                for m in range(4):
                    nc.scalar.activation(v1_8[:, m // 2, m % 2, :], psC[m][:], Relu,
                                         bias=bcol(2, m))

                # ---- ct1 (fp8 DR): v2 in frame S_WC2 ----
                psD = [pspool.tile([128, NT], f32, tag="ps", name=f"psD_{n}_{m}")
                       for m in range(4)]
                v2_8 = hpool.tile([128, 2, 2, NT], f8, tag="v2", name=f"v2_{n}")
                for k in range(2):
                    for m in range(4):
                        nc.tensor.matmul(psD[m][:], wc2[:, k, :, 128 * m:128 * (m + 1)],
                                         t_8[:, k, :, :], start=k == 0, stop=k == 1,
                                         perf_mode=DR)
                for m in range(4):
                    nc.scalar.activation(v2_8[:, m // 2, m % 2, :], psD[m][:], Relu,
                                         bias=bcol(3, m))

                # ---- V = v1 @ Mv.T (fp8 DR) + i_ @ fi2s.T (bf16), shared frame ----
                psV = [pspool.tile([128, NT], f32, tag="ps", name=f"psV_{n}_{m}")
                       for m in range(4)]
                for m in range(4):
                    for k in range(2):
                        nc.tensor.matmul(psV[m][:], wV1[:, k, :, 128 * m:128 * (m + 1)],
                                         v1_8[:, k, :, :], start=k == 0, stop=False,
                                         perf_mode=DR)
                    for k in range(4):
                        nc.tensor.matmul(psV[m][:], wf2[:, k, 128 * m:128 * (m + 1)],
                                         i_b[:, k, :], start=False, stop=k == 3)
                    oV = opool.tile([128, NT], bf16, tag="o", name=f"oV_{n}_{m}")
                    nc.scalar.activation(oV[:], psV[m][:], Ident,
                                         bias=bcol(4, m), scale=scales["V"])
                    nc.sync.dma_start(out_d[128 * m:128 * (m + 1), c0:c0 + NT], oV[:])

                # ---- T = v2 @ Mt.T + t_ @ ft2.T (both fp8 DR, shared frame) ----
                psT = [pspool.tile([128, NT], f32, tag="ps", name=f"psT_{n}_{m}")
                       for m in range(4)]
                for m in range(4):
                    for k in range(2):
                        nc.tensor.matmul(psT[m][:], wT2[:, k, :, 128 * m:128 * (m + 1)],
                                         v2_8[:, k, :, :], start=k == 0, stop=False,
                                         perf_mode=DR)
                    for k in range(2):
                        nc.tensor.matmul(psT[m][:], wTt[:, k, :, 128 * m:128 * (m + 1)],
                                         t_8[:, k, :, :], start=False, stop=k == 1,
                                         perf_mode=DR)
                    oT = opool.tile([128, NT], bf16, tag="o", name=f"oT_{n}_{m}")
                    nc.scalar.activation(oT[:], psT[m][:], Ident,
                                         bias=bcol(5, m), scale=scales["T"])
                    nc.sync.dma_start(
                        out_d[HID + 128 * m:HID + 128 * (m + 1), c0:c0 + NT], oT[:])

                if n + 1 < ntiles:
                    x_cur = x_nxt
                    xt_cur = xt_nxt

    nc.compile()
    return nc


def kernel(**inputs) -> np.ndarray:
    from concourse import bass_utils

    i = np.asarray(inputs["i"], dtype=np.float32)
    t = np.asarray(inputs["t"], dtype=np.float32)
    weights, scales = _host_pack(inputs)

    key = ("nc", tuple(round(v, 12) for v in sorted(scales.values())))
    if key not in _CACHE:
        _CACHE[key] = _build_nc(BS, scales)
    nc = _CACHE[key]

    iT = np.ascontiguousarray(i.T).astype(BF)   # [IMG, B]
    tT = np.ascontiguousarray(t.T).astype(BF)   # [TAB, B]

    in_maps = []
    for c in range(NCORES):
        sl = slice(c * BS, (c + 1) * BS)
        m = dict(weights)
        m["iT"] = np.ascontiguousarray(iT[:, sl])
        m["tT"] = np.ascontiguousarray(tT[:, sl])
        in_maps.append(m)

    res = bass_utils.run_bass_kernel_spmd(nc, in_maps, core_ids=list(range(NCORES)))

    out = np.empty((B, 2 * HID), dtype=np.float32)
    for c in range(NCORES):
        out[c * BS:(c + 1) * BS] = res.results[c]["outT"].T.astype(np.float32)
    return out
